# revision 45
# baseline (speedup 1.0000x reference)
"""MoE grouped-GEMM (8 experts) on 8 Trainium2 NeuronCores.

Problem: input [32768, 1024] routed contiguously to 8 experts (counts in
num_experts_per_token); expert i computes x_i @ W_i.T + b_i with
W [8, 4096, 1024], b [8, 4096]. Output [32768, 4096].

Sharding: expert-parallel, expert i <-> core i. Zero collectives: the host
slices each expert's token block, packs x and W into SBUF tile layout
(contraction dim DIN on partitions), runs a 4096x1024x4096 GEMM (+bias) per
core, and concatenates per-core outputs.

All-bf16 device kernel (same 1 cycle/row PE rate as float32r, but half
the DMA/SBUF and FWL 2x-faster weight loads); steady state runs at the
N=512 streaming floor (median matmul issue gap 216 ns = 512/2.4 GHz +
NX overhead; ~463 us vs the 442 us pure-streaming floor per core):
  - everything is SBUF-resident (single phase; no x re-streaming)
  - chunk 0 runs n-outer/k-inner gated on 128 KB k-slice tiles of x and
    the n=0 weights (deps are per-tile, so gate tiles must be separate);
    5 dummy matmuls on memset tiles bridge the DMA-starved window after
    the ~7 us framework preamble so HAM reaches 2.4 GHz with one
    transition
  - chunks 1-7 run m-outer/k-mid/n-inner: the stationary x-tile is
    reused across 8 matmuls into 8 parallel PSUM-bank accumulators
  - the 7 MB x-chunk stream and late bias slices are deferred behind a
    chunk-0 drain (add_dep_helper) so the latency-critical weight stream
    owns early HBM bandwidth
  - DVE fuses bias-add with the PSUM drain, writing bf16; output DMAs
    are full-row 1 MB transfers on the sync ring; the last m-tile drains
    n-outer with per-n 64 KB DMAs so the kernel tail is ~5 us
  - host upcasts the bf16 output to fp32 (rel err ~3e-3 << 2e-2 gate)
"""

import sys

if "/opt/trn_rl_repo" not in sys.path:
    sys.path.insert(0, "/opt/trn_rl_repo")

import numpy as np
from ml_dtypes import bfloat16

E, T, DIN, DOUT = 8, 32768, 1024, 4096
NCORES = 8
TOKC = T // NCORES  # tokens per core (capacity)

KT = 128   # contraction tile (SBUF partitions)
MT = 128   # token tile (PSUM partitions)
NT = 512   # dout tile (one fp32 PSUM bank)
KTILES = DIN // KT    # 8
NTILES = DOUT // NT   # 8

CT = 512                 # token chunk
CHUNKS = TOKC // CT      # 8
CMT = CT // MT           # 4 token tiles per chunk

_CACHE = {}


def _build_nc():
    import concourse.bacc as bacc
    import concourse.tile as tile
    import concourse.mybir as mybir
    from concourse.tile import add_dep_helper

    nc = bacc.Bacc("TRN2", target_bir_lowering=False, debug=False,
                   num_devices=NCORES)
    bf = mybir.dt.bfloat16
    f32 = mybir.dt.float32

    # chunk-0 x and n=0 weights arrive as 128 KB k-slices. Fine granularity
    # is load-bearing: HBM is shared round-robin across all in-flight DMAs
    # and a DMA completes only when its LAST byte lands, so small slices
    # give progressive early data while big blocks complete late (measured:
    # 256 KB pairs and a single 896 KB rest-block both regressed ~4 us)
    x0P = nc.dram_tensor("x0P", [KTILES, KT, CT], bf, kind="ExternalInput")
    w0P = nc.dram_tensor("w0P", [KTILES, KT, NT], bf, kind="ExternalInput")
    xP = nc.dram_tensor("xP", [CHUNKS - 1, KT, KTILES * CT], bf,
                        kind="ExternalInput")
    wP = nc.dram_tensor("wP", [NTILES - 1, KT, KTILES * NT], bf,
                        kind="ExternalInput")
    biasP = nc.dram_tensor("biasP", [NTILES, MT, NT], f32,
                           kind="ExternalInput")
    y = nc.dram_tensor("y", [TOKC, DOUT], bf, kind="ExternalOutput")

    with tile.TileContext(nc) as tc:
        with (
            tc.tile_pool(name="wpool", bufs=1) as wpool,
            tc.tile_pool(name="xpool", bufs=1) as xpool,
            tc.tile_pool(name="bpool", bufs=1) as bpool,
            tc.tile_pool(name="o0pool", bufs=1) as o0pool,
            tc.tile_pool(name="opool", bufs=2) as opool,
            tc.tile_pool(name="psum", bufs=8, space="PSUM") as psum_pool,
        ):
            # HAM warmup: dummy matmuls on memset tiles bridge the
            # DMA-starved gate window (~8-11 us) so real matmuls start at
            # the 2.4 GHz clock instead of paying the 3.4 us cold ramp
            dumx = xpool.tile([KT, MT], bf, name="dumx", tag="dumx")
            dumw = wpool.tile([KT, NT], bf, name="dumw", tag="dumw")
            nc.gpsimd.memset(dumx[:], 0)
            nc.gpsimd.memset(dumw[:], 0)
            dacc = psum_pool.tile([MT, NT], mybir.dt.float32, name="acc",
                                  tag="acc")
            for _ in range(5):
                nc.tensor.matmul(dacc[:], dumx[:], dumw[:],
                                 start=True, stop=True)

            # chunk-0 x k-slices as separate 128 KB tiles: the first matmul
            # gates on one slice, not the whole chunk (deps are per-tile)
            x0k = [xpool.tile([KT, CT], bf, name=f"x0k{k}", tag=f"x0k{k}")
                   for k in range(KTILES)]
            for k in range(KTILES):
                nc.scalar.dma_start(x0k[k][:], x0P[k])

            def x0s(k, m):  # stationary [128, MT] slice for chunk 0
                return x0k[k][:, m * MT:(m + 1) * MT]

            # n=0 weights k-sliced on the sync ring (gates chunk 0);
            # n=1..7 as whole 1 MB tiles behind them
            w0k = [wpool.tile([KT, NT], bf, name=f"w0k{k}", tag=f"w0k{k}")
                   for k in range(KTILES)]
            for k in range(KTILES):
                nc.sync.dma_start(w0k[k][:], w0P[k])
            wt = [wpool.tile([KT, KTILES * NT], bf, name=f"wt{n}",
                             tag=f"wt{n}") for n in range(1, NTILES)]
            for n in range(1, NTILES):
                nc.sync.dma_start(wt[n - 1][:], wP[n - 1])

            def ws(n, k):  # moving [128, NT] slice of expert weights
                if n == 0:
                    return w0k[k][:]
                return wt[n - 1][:, k * NT:(k + 1) * NT]

            # bias as per-n slices on the scalar ring BEHIND the x0k gate
            # slices. Only bias[0] loads eagerly (needed at the first drain
            # ~16 us); bias[1..7] are deferred below so the gate window
            # carries nothing but x0k + bias0 + the w stream
            bias_n = [bpool.tile([MT, NT], f32, name=f"bias{n}",
                                 tag=f"bias{n}") for n in range(NTILES)]
            nc.scalar.dma_start(bias_n[0][:], biasP[0])

            # chunks 1-7 tiles; their DMAs are deferred (emitted after the
            # chunk-0 gate drain below) so the 7 MB x stream doesn't steal
            # HBM bandwidth from the latency-critical weight stream
            xt = [xpool.tile([KT, KTILES * CT], bf, name=f"xt{c}",
                             tag=f"xt{c}") for c in range(1, CHUNKS)]

            # ---- chunk 0: n-outer / k-inner, gated by wt[n] arrival ----
            o0 = [o0pool.tile([MT, DOUT], bf, name=f"o0_{m}", tag=f"o0_{m}")
                  for m in range(CMT)]
            xt_gate = None
            bias_gate = None
            for n in range(NTILES):
                for m in range(CMT):
                    acc = psum_pool.tile([MT, NT], f32, name="acc", tag="acc")
                    for k in range(KTILES):
                        nc.tensor.matmul(
                            acc[:], x0s(k, m), ws(n, k),
                            start=(k == 0), stop=(k == KTILES - 1))
                    tt = nc.vector.tensor_add(
                        o0[m][:, n * NT:(n + 1) * NT], acc[:],
                        bias_n[n][:])
                    if n == 0 and m == 1:
                        bias_gate = tt
                    if n == 2 and m == CMT - 1:
                        xt_gate = tt
            for m in range(CMT):
                nc.sync.dma_start(y[m * MT:(m + 1) * MT, :], o0[m][:])

            # bias[1..3] land ~20 us (needed ~23 us at the n=1 drains),
            # clearing another 768 KB out of the gate window
            for n in range(1, 4):
                d = nc.scalar.dma_start(bias_n[n][:], biasP[n])
                add_dep_helper(d.ins, bias_gate.ins,
                               reason="defer mid bias behind w stream")

            # release the x chunk stream (and the late bias slices) once the
            # weight stream has had the HBM to itself (~30 us in, vs first
            # need at ~41 us for bias4 and ~67 us for xt[0])
            for n in range(4, NTILES):
                d = nc.scalar.dma_start(bias_n[n][:], biasP[n])
                add_dep_helper(d.ins, xt_gate.ins,
                               reason="defer late bias behind w stream")
            for i in range(CHUNKS - 1):
                d = nc.scalar.dma_start(xt[i][:], xP[i])
                add_dep_helper(d.ins, xt_gate.ins,
                               reason="defer x stream behind w stream")

            # ---- chunks 1-7: m-outer / k-mid / n-inner ----
            # stationary x[k,m] is shared by 8 matmuls into 8 PSUM banks
            for c in range(1, CHUNKS):
                xc = xt[c - 1]
                for m in range(CMT):
                    last_tile = c == CHUNKS - 1 and m == CMT - 1
                    row0 = c * CT + m * MT
                    if last_tile:
                        # n-outer so each bank drains right after its own
                        # k-group, with per-n 64 KB output DMAs: the kernel
                        # tail is one TT + one small DMA, not 8 TTs + 1 MB
                        ot = opool.tile([MT, DOUT], bf, name="ot", tag="ot")
                        for n in range(NTILES):
                            acc = psum_pool.tile([MT, NT], f32, name="acc",
                                                 tag="acc")
                            for k in range(KTILES):
                                xs = xc[:, k * CT + m * MT:
                                        k * CT + (m + 1) * MT]
                                nc.tensor.matmul(
                                    acc[:], xs, ws(n, k),
                                    start=(k == 0), stop=(k == KTILES - 1))
                            nc.vector.tensor_add(
                                ot[:, n * NT:(n + 1) * NT], acc[:],
                                bias_n[n][:])
                            nc.sync.dma_start(
                                y[row0:row0 + MT, n * NT:(n + 1) * NT],
                                ot[:, n * NT:(n + 1) * NT])
                        continue
                    accs = [psum_pool.tile([MT, NT], f32, name="acc",
                                           tag="acc") for _ in range(NTILES)]
                    for k in range(KTILES):
                        xs = xc[:, k * CT + m * MT:k * CT + (m + 1) * MT]
                        for n in range(NTILES):
                            nc.tensor.matmul(
                                accs[n][:], xs, ws(n, k),
                                start=(k == 0), stop=(k == KTILES - 1))
                    ot = opool.tile([MT, DOUT], bf, name="ot", tag="ot")
                    for n in range(NTILES):
                        nc.vector.tensor_add(
                            ot[:, n * NT:(n + 1) * NT], accs[n][:],
                            bias_n[n][:])
                    nc.sync.dma_start(y[row0:row0 + MT, :], ot[:])

    nc.compile()
    return nc


def _install_neff_cache():
    """Disk-cache walrus NEFF compiles keyed on the BIR bytes."""
    if _CACHE.get("neff_cache_installed"):
        return
    _CACHE["neff_cache_installed"] = True
    import hashlib
    import os
    import shutil

    import concourse.bass2jax as bass2jax

    cache_dir = "/root/.neff_bir_cache"
    os.makedirs(cache_dir, exist_ok=True)
    orig = bass2jax.compile_bir_kernel

    def cached_compile(ant_bir_str, tmpdir, neff_name="file.neff", **kw):
        key = hashlib.sha256(
            ant_bir_str if isinstance(ant_bir_str, bytes)
            else ant_bir_str.encode()).hexdigest()
        hit = os.path.join(cache_dir, key + ".neff")
        dst = os.path.join(tmpdir, neff_name)
        if os.path.exists(hit):
            shutil.copyfile(hit, dst)
            return dst
        out = orig(ant_bir_str, tmpdir, neff_name=neff_name, **kw)
        try:
            shutil.copyfile(out, hit)
        except OSError:
            pass
        return out

    bass2jax.compile_bir_kernel = cached_compile


def _get_nc():
    if "nc" not in _CACHE:
        _install_neff_cache()
        _CACHE["nc"] = _build_nc()
    return _CACHE["nc"]


def _pack_blocks(a2d, blocks, inner):
    """[blocks*inner, K*128] -> [blocks, 128, K*inner]:
    out[b, p, k*inner + j] = a2d[b*inner + j, k*128 + p]."""
    rows, cols = a2d.shape
    kb = cols // KT
    return np.ascontiguousarray(
        a2d.reshape(blocks, inner, kb, KT).transpose(0, 3, 2, 1)
        .reshape(blocks, KT, kb * inner))


def kernel(input, weight, bias, num_experts_per_token):
    from concourse.bass_utils import run_bass_kernel_spmd

    input = np.asarray(input, dtype=np.float32)
    weight = np.asarray(weight, dtype=np.float32)
    bias = np.ascontiguousarray(np.asarray(bias, dtype=np.float32))
    counts = np.asarray(num_experts_per_token).astype(np.int64)
    offsets = np.concatenate([[0], np.cumsum(counts)]).astype(np.int64)

    if counts.max() > TOKC:
        # capacity overflow (never hit with balanced routing): numpy fallback
        outs = []
        for i in range(E):
            xi = input[offsets[i]:offsets[i + 1]]
            outs.append(xi @ weight[i].T + bias[i])
        return np.concatenate(outs, axis=0)

    in_maps = []
    for i in range(E):
        xi = input[offsets[i]:offsets[i + 1]]  # [n_i, DIN]
        if xi.shape[0] < TOKC:
            xi = np.concatenate(
                [xi, np.zeros((TOKC - xi.shape[0], DIN), np.float32)], axis=0)
        xall = _pack_blocks(xi.astype(bfloat16), CHUNKS, CT)  # [8, 128, 4096]
        # chunk-0 cols are k*CT + j, so a k-slice is contiguous cols
        x0 = np.ascontiguousarray(
            xall[0].reshape(KT, KTILES, CT).transpose(1, 0, 2))
        wp = _pack_blocks(weight[i].astype(bfloat16), NTILES, NT)
        w0 = np.ascontiguousarray(
            wp[0].reshape(KT, KTILES, NT).transpose(1, 0, 2))
        bb = np.ascontiguousarray(
            np.broadcast_to(bias[i][None, :], (MT, DOUT))
            .reshape(MT, NTILES, NT).transpose(1, 0, 2))
        in_maps.append({"x0P": x0, "xP": np.ascontiguousarray(xall[1:]),
                        "w0P": w0, "wP": np.ascontiguousarray(wp[1:]),
                        "biasP": bb})

    nc = _get_nc()
    import os
    trace = bool(int(os.environ.get("KERNEL_TRACE", "0")))
    res = run_bass_kernel_spmd(nc, in_maps, core_ids=list(range(NCORES)),
                               trace=trace)
    _CACHE["last_result"] = res

    out = np.empty((T, DOUT), dtype=np.float32)
    pos = 0
    for i in range(E):
        n_i = int(counts[i])
        out[pos:pos + n_i] = res.results[i]["y"][:n_i].astype(np.float32)
        pos += n_i
    return out


# revision 47
# speedup vs baseline: 1.0127x; 1.0127x over previous
"""MoE grouped-GEMM (8 experts) on 8 Trainium2 NeuronCores.

Problem: input [32768, 1024] routed contiguously to 8 experts (counts in
num_experts_per_token); expert i computes x_i @ W_i.T + b_i with
W [8, 4096, 1024], b [8, 4096]. Output [32768, 4096].

Sharding: expert-parallel, expert i <-> core i. Zero collectives: the host
slices each expert's token block, packs x and W into SBUF tile layout
(contraction dim DIN on partitions), runs a 4096x1024x4096 GEMM (+bias) per
core, and concatenates per-core outputs.

All-bf16 device kernel (same 1 cycle/row PE rate as float32r, but half
the DMA/SBUF and FWL 2x-faster weight loads); steady state runs at the
N=512 streaming floor (median matmul issue gap 216 ns = 512/2.4 GHz +
NX overhead; ~463 us vs the 442 us pure-streaming floor per core):
  - everything is SBUF-resident (single phase; no x re-streaming)
  - chunk 0 runs n-outer/k-inner gated on 128 KB k-slice tiles of x and
    the n=0 weights (deps are per-tile, so gate tiles must be separate);
    5 dummy matmuls on memset tiles bridge the DMA-starved window after
    the ~7 us framework preamble so HAM reaches 2.4 GHz with one
    transition
  - chunks 1-7 run m-outer/k-mid/n-inner: the stationary x-tile is
    reused across 8 matmuls into 8 parallel PSUM-bank accumulators
  - the 7 MB x-chunk stream and late bias slices are deferred behind a
    chunk-0 drain (add_dep_helper) so the latency-critical weight stream
    owns early HBM bandwidth
  - DVE fuses bias-add with the PSUM drain, writing bf16; output DMAs
    are full-row 1 MB transfers on the sync ring; the last m-tile drains
    n-outer with per-n 64 KB DMAs so the kernel tail is ~5 us
  - host upcasts the bf16 output to fp32 (rel err ~3e-3 << 2e-2 gate)
"""

import sys

if "/opt/trn_rl_repo" not in sys.path:
    sys.path.insert(0, "/opt/trn_rl_repo")

import numpy as np
from ml_dtypes import bfloat16

E, T, DIN, DOUT = 8, 32768, 1024, 4096
NCORES = 8
TOKC = T // NCORES  # tokens per core (capacity)

KT = 128   # contraction tile (SBUF partitions)
MT = 128   # token tile (PSUM partitions)
NT = 512   # dout tile (one fp32 PSUM bank)
KTILES = DIN // KT    # 8
NTILES = DOUT // NT   # 8

CT = 512                 # token chunk
CHUNKS = TOKC // CT      # 8
CMT = CT // MT           # 4 token tiles per chunk

_CACHE = {}


def _build_nc():
    import concourse.bacc as bacc
    import concourse.tile as tile
    import concourse.mybir as mybir
    from concourse.tile import add_dep_helper

    nc = bacc.Bacc("TRN2", target_bir_lowering=False, debug=False,
                   num_devices=NCORES)
    bf = mybir.dt.bfloat16
    f32 = mybir.dt.float32

    # chunk-0 x and n=0 weights arrive as 128 KB k-slices. Fine granularity
    # is load-bearing: HBM is shared round-robin across all in-flight DMAs
    # and a DMA completes only when its LAST byte lands, so small slices
    # give progressive early data while big blocks complete late (measured:
    # 256 KB pairs and a single 896 KB rest-block both regressed ~4 us)
    x0P = nc.dram_tensor("x0P", [KTILES, KT, CT], bf, kind="ExternalInput")
    w0P = nc.dram_tensor("w0P", [KTILES, KT, NT], bf, kind="ExternalInput")
    xP = nc.dram_tensor("xP", [CHUNKS - 1, KT, KTILES * CT], bf,
                        kind="ExternalInput")
    wP = nc.dram_tensor("wP", [NTILES - 1, KT, KTILES * NT], bf,
                        kind="ExternalInput")
    biasP = nc.dram_tensor("biasP", [NTILES, MT, NT], f32,
                           kind="ExternalInput")
    y = nc.dram_tensor("y", [TOKC, DOUT], bf, kind="ExternalOutput")

    with tile.TileContext(nc) as tc:
        with (
            tc.tile_pool(name="wpool", bufs=1) as wpool,
            tc.tile_pool(name="xpool", bufs=1) as xpool,
            tc.tile_pool(name="bpool", bufs=1) as bpool,
            tc.tile_pool(name="o0pool", bufs=1) as o0pool,
            tc.tile_pool(name="opool", bufs=2) as opool,
            tc.tile_pool(name="psum", bufs=8, space="PSUM") as psum_pool,
        ):
            # HAM warmup: dummy matmuls on memset tiles bridge the
            # DMA-starved gate window (~8-11 us) so real matmuls start at
            # the 2.4 GHz clock instead of paying the 3.4 us cold ramp
            dumx = xpool.tile([KT, MT], bf, name="dumx", tag="dumx")
            dumw = wpool.tile([KT, NT], bf, name="dumw", tag="dumw")
            nc.gpsimd.memset(dumx[:], 0)
            nc.gpsimd.memset(dumw[:], 0)
            dacc = psum_pool.tile([MT, NT], mybir.dt.float32, name="acc",
                                  tag="acc")
            for _ in range(5):
                nc.tensor.matmul(dacc[:], dumx[:], dumw[:],
                                 start=True, stop=True)

            # chunk-0 x k-slices as separate 128 KB tiles: the first matmul
            # gates on one slice, not the whole chunk (deps are per-tile)
            x0k = [xpool.tile([KT, CT], bf, name=f"x0k{k}", tag=f"x0k{k}")
                   for k in range(KTILES)]
            for k in range(KTILES):
                nc.scalar.dma_start(x0k[k][:], x0P[k])

            def x0s(k, m):  # stationary [128, MT] slice for chunk 0
                return x0k[k][:, m * MT:(m + 1) * MT]

            # n=0 weights k-sliced on the sync ring (gates chunk 0);
            # n=1..7 as whole 1 MB tiles behind them
            w0k = [wpool.tile([KT, NT], bf, name=f"w0k{k}", tag=f"w0k{k}")
                   for k in range(KTILES)]
            for k in range(KTILES):
                nc.sync.dma_start(w0k[k][:], w0P[k])
            wt = [wpool.tile([KT, KTILES * NT], bf, name=f"wt{n}",
                             tag=f"wt{n}") for n in range(1, NTILES)]
            for n in range(1, NTILES):
                nc.sync.dma_start(wt[n - 1][:], wP[n - 1])

            def ws(n, k):  # moving [128, NT] slice of expert weights
                if n == 0:
                    return w0k[k][:]
                return wt[n - 1][:, k * NT:(k + 1) * NT]

            # bias as per-n slices on the scalar ring BEHIND the x0k gate
            # slices: each drain gates on its own 256 KB slice, and the
            # early HBM window stays with the w stream
            bias_n = [bpool.tile([MT, NT], f32, name=f"bias{n}",
                                 tag=f"bias{n}") for n in range(NTILES)]
            for n in range(4):
                nc.scalar.dma_start(bias_n[n][:], biasP[n])

            # chunks 1-7 tiles; their DMAs are deferred (emitted after the
            # chunk-0 gate drain below) so the 7 MB x stream doesn't steal
            # HBM bandwidth from the latency-critical weight stream
            xt = [xpool.tile([KT, KTILES * CT], bf, name=f"xt{c}",
                             tag=f"xt{c}") for c in range(1, CHUNKS)]

            # ---- chunk 0: n-outer / k-inner, gated by wt[n] arrival ----
            o0 = [o0pool.tile([MT, DOUT], bf, name=f"o0_{m}", tag=f"o0_{m}")
                  for m in range(CMT)]
            xt_gate = None
            for n in range(NTILES):
                for m in range(CMT):
                    acc = psum_pool.tile([MT, NT], f32, name="acc", tag="acc")
                    for k in range(KTILES):
                        nc.tensor.matmul(
                            acc[:], x0s(k, m), ws(n, k),
                            start=(k == 0), stop=(k == KTILES - 1))
                    tt = nc.vector.tensor_add(
                        o0[m][:, n * NT:(n + 1) * NT], acc[:],
                        bias_n[n][:])
                    if n == 2 and m == CMT - 1:
                        xt_gate = tt
            for m in range(CMT):
                nc.sync.dma_start(y[m * MT:(m + 1) * MT, :], o0[m][:])

            # release the x chunk stream (and the late bias slices) once the
            # weight stream has had the HBM to itself (~30 us in, vs first
            # need at ~41 us for bias4 and ~67 us for xt[0])
            for n in range(4, NTILES):
                d = nc.scalar.dma_start(bias_n[n][:], biasP[n])
                add_dep_helper(d.ins, xt_gate.ins,
                               reason="defer late bias behind w stream")
            for i in range(CHUNKS - 1):
                d = nc.scalar.dma_start(xt[i][:], xP[i])
                add_dep_helper(d.ins, xt_gate.ins,
                               reason="defer x stream behind w stream")

            # ---- chunks 1-7: m-outer / k-mid / n-inner ----
            # stationary x[k,m] is shared by 8 matmuls into 8 PSUM banks
            for c in range(1, CHUNKS):
                xc = xt[c - 1]
                for m in range(CMT):
                    last_tile = c == CHUNKS - 1 and m == CMT - 1
                    row0 = c * CT + m * MT
                    if last_tile:
                        # n-outer so each bank drains right after its own
                        # k-group, with per-n 64 KB output DMAs: the kernel
                        # tail is one TT + one small DMA, not 8 TTs + 1 MB
                        ot = opool.tile([MT, DOUT], bf, name="ot", tag="ot")
                        for n in range(NTILES):
                            acc = psum_pool.tile([MT, NT], f32, name="acc",
                                                 tag="acc")
                            for k in range(KTILES):
                                xs = xc[:, k * CT + m * MT:
                                        k * CT + (m + 1) * MT]
                                nc.tensor.matmul(
                                    acc[:], xs, ws(n, k),
                                    start=(k == 0), stop=(k == KTILES - 1))
                            nc.vector.tensor_add(
                                ot[:, n * NT:(n + 1) * NT], acc[:],
                                bias_n[n][:])
                            nc.sync.dma_start(
                                y[row0:row0 + MT, n * NT:(n + 1) * NT],
                                ot[:, n * NT:(n + 1) * NT])
                        continue
                    accs = [psum_pool.tile([MT, NT], f32, name="acc",
                                           tag="acc") for _ in range(NTILES)]
                    for k in range(KTILES):
                        xs = xc[:, k * CT + m * MT:k * CT + (m + 1) * MT]
                        for n in range(NTILES):
                            nc.tensor.matmul(
                                accs[n][:], xs, ws(n, k),
                                start=(k == 0), stop=(k == KTILES - 1))
                    ot = opool.tile([MT, DOUT], bf, name="ot", tag="ot")
                    for n in range(NTILES):
                        nc.vector.tensor_add(
                            ot[:, n * NT:(n + 1) * NT], accs[n][:],
                            bias_n[n][:])
                    nc.sync.dma_start(y[row0:row0 + MT, :], ot[:])

    nc.compile()
    return nc


def _install_neff_cache():
    """Disk-cache walrus NEFF compiles keyed on the BIR bytes."""
    if _CACHE.get("neff_cache_installed"):
        return
    _CACHE["neff_cache_installed"] = True
    import hashlib
    import os
    import shutil

    import concourse.bass2jax as bass2jax

    cache_dir = "/root/.neff_bir_cache"
    os.makedirs(cache_dir, exist_ok=True)
    orig = bass2jax.compile_bir_kernel

    def cached_compile(ant_bir_str, tmpdir, neff_name="file.neff", **kw):
        key = hashlib.sha256(
            ant_bir_str if isinstance(ant_bir_str, bytes)
            else ant_bir_str.encode()).hexdigest()
        hit = os.path.join(cache_dir, key + ".neff")
        dst = os.path.join(tmpdir, neff_name)
        if os.path.exists(hit):
            shutil.copyfile(hit, dst)
            return dst
        out = orig(ant_bir_str, tmpdir, neff_name=neff_name, **kw)
        try:
            shutil.copyfile(out, hit)
        except OSError:
            pass
        return out

    bass2jax.compile_bir_kernel = cached_compile


def _get_nc():
    if "nc" not in _CACHE:
        _install_neff_cache()
        _CACHE["nc"] = _build_nc()
    return _CACHE["nc"]


def _pack_blocks(a2d, blocks, inner):
    """[blocks*inner, K*128] -> [blocks, 128, K*inner]:
    out[b, p, k*inner + j] = a2d[b*inner + j, k*128 + p]."""
    rows, cols = a2d.shape
    kb = cols // KT
    return np.ascontiguousarray(
        a2d.reshape(blocks, inner, kb, KT).transpose(0, 3, 2, 1)
        .reshape(blocks, KT, kb * inner))


def kernel(input, weight, bias, num_experts_per_token):
    from concourse.bass_utils import run_bass_kernel_spmd

    input = np.asarray(input, dtype=np.float32)
    weight = np.asarray(weight, dtype=np.float32)
    bias = np.ascontiguousarray(np.asarray(bias, dtype=np.float32))
    counts = np.asarray(num_experts_per_token).astype(np.int64)
    offsets = np.concatenate([[0], np.cumsum(counts)]).astype(np.int64)

    if counts.max() > TOKC:
        # capacity overflow (never hit with balanced routing): numpy fallback
        outs = []
        for i in range(E):
            xi = input[offsets[i]:offsets[i + 1]]
            outs.append(xi @ weight[i].T + bias[i])
        return np.concatenate(outs, axis=0)

    in_maps = []
    for i in range(E):
        xi = input[offsets[i]:offsets[i + 1]]  # [n_i, DIN]
        if xi.shape[0] < TOKC:
            xi = np.concatenate(
                [xi, np.zeros((TOKC - xi.shape[0], DIN), np.float32)], axis=0)
        xall = _pack_blocks(xi.astype(bfloat16), CHUNKS, CT)  # [8, 128, 4096]
        # chunk-0 cols are k*CT + j, so a k-slice is contiguous cols
        x0 = np.ascontiguousarray(
            xall[0].reshape(KT, KTILES, CT).transpose(1, 0, 2))
        wp = _pack_blocks(weight[i].astype(bfloat16), NTILES, NT)
        w0 = np.ascontiguousarray(
            wp[0].reshape(KT, KTILES, NT).transpose(1, 0, 2))
        bb = np.ascontiguousarray(
            np.broadcast_to(bias[i][None, :], (MT, DOUT))
            .reshape(MT, NTILES, NT).transpose(1, 0, 2))
        in_maps.append({"x0P": x0, "xP": np.ascontiguousarray(xall[1:]),
                        "w0P": w0, "wP": np.ascontiguousarray(wp[1:]),
                        "biasP": bb})

    nc = _get_nc()
    import os
    trace = bool(int(os.environ.get("KERNEL_TRACE", "0")))
    res = run_bass_kernel_spmd(nc, in_maps, core_ids=list(range(NCORES)),
                               trace=trace)
    _CACHE["last_result"] = res

    out = np.empty((T, DOUT), dtype=np.float32)
    pos = 0
    for i in range(E):
        n_i = int(counts[i])
        out[pos:pos + n_i] = res.results[i]["y"][:n_i].astype(np.float32)
        pos += n_i
    return out


# revision 48
# speedup vs baseline: 1.0157x; 1.0030x over previous
"""MoE grouped-GEMM (8 experts) on 8 Trainium2 NeuronCores.

Problem: input [32768, 1024] routed contiguously to 8 experts (counts in
num_experts_per_token); expert i computes x_i @ W_i.T + b_i with
W [8, 4096, 1024], b [8, 4096]. Output [32768, 4096].

Sharding: expert-parallel, expert i <-> core i. Zero collectives: the host
slices each expert's token block, packs x and W into SBUF tile layout
(contraction dim DIN on partitions), runs a 4096x1024x4096 GEMM (+bias) per
core, and concatenates per-core outputs.

All-bf16 device kernel (same 1 cycle/row PE rate as float32r, but half
the DMA/SBUF and FWL 2x-faster weight loads); steady state runs at the
N=512 streaming floor (median matmul issue gap 216 ns = 512/2.4 GHz +
NX overhead; ~463 us vs the 442 us pure-streaming floor per core):
  - everything is SBUF-resident (single phase; no x re-streaming)
  - chunk 0 runs n-outer/k-inner gated on 128 KB k-slice tiles of x and
    the n=0 weights (deps are per-tile, so gate tiles must be separate);
    5 dummy matmuls on memset tiles bridge the DMA-starved window after
    the ~7 us framework preamble so HAM reaches 2.4 GHz with one
    transition
  - chunks 1-7 run m-outer/k-mid/n-inner: the stationary x-tile is
    reused across 8 matmuls into 8 parallel PSUM-bank accumulators
  - the 7 MB x-chunk stream and late bias slices are deferred behind a
    chunk-0 drain (add_dep_helper) so the latency-critical weight stream
    owns early HBM bandwidth
  - DVE fuses bias-add with the PSUM drain, writing bf16; output DMAs
    are full-row 1 MB transfers on the sync ring; the last m-tile drains
    n-outer with per-n 64 KB DMAs so the kernel tail is ~5 us
  - host upcasts the bf16 output to fp32 (rel err ~3e-3 << 2e-2 gate)
"""

import sys

if "/opt/trn_rl_repo" not in sys.path:
    sys.path.insert(0, "/opt/trn_rl_repo")

import numpy as np
from ml_dtypes import bfloat16

E, T, DIN, DOUT = 8, 32768, 1024, 4096
NCORES = 8
TOKC = T // NCORES  # tokens per core (capacity)

KT = 128   # contraction tile (SBUF partitions)
MT = 128   # token tile (PSUM partitions)
NT = 512   # dout tile (one fp32 PSUM bank)
KTILES = DIN // KT    # 8
NTILES = DOUT // NT   # 8

CT = 512                 # token chunk
CHUNKS = TOKC // CT      # 8
CMT = CT // MT           # 4 token tiles per chunk

_CACHE = {}


def _build_nc():
    import concourse.bacc as bacc
    import concourse.tile as tile
    import concourse.mybir as mybir
    from concourse.tile import add_dep_helper

    nc = bacc.Bacc("TRN2", target_bir_lowering=False, debug=False,
                   num_devices=NCORES)
    bf = mybir.dt.bfloat16
    f32 = mybir.dt.float32

    # chunk-0 x and n=0 weights arrive as 128 KB k-slices. Fine granularity
    # is load-bearing: HBM is shared round-robin across all in-flight DMAs
    # and a DMA completes only when its LAST byte lands, so small slices
    # give progressive early data while big blocks complete late (measured:
    # 256 KB pairs and a single 896 KB rest-block both regressed ~4 us)
    x0P = nc.dram_tensor("x0P", [KTILES, KT, CT], bf, kind="ExternalInput")
    w0P = nc.dram_tensor("w0P", [KTILES, KT, NT], bf, kind="ExternalInput")
    xP = nc.dram_tensor("xP", [CHUNKS - 1, KT, KTILES * CT], bf,
                        kind="ExternalInput")
    wP = nc.dram_tensor("wP", [NTILES - 1, KT, KTILES * NT], bf,
                        kind="ExternalInput")
    biasP = nc.dram_tensor("biasP", [NTILES, MT, NT], f32,
                           kind="ExternalInput")
    y = nc.dram_tensor("y", [TOKC, DOUT], bf, kind="ExternalOutput")

    with tile.TileContext(nc) as tc:
        with (
            tc.tile_pool(name="wpool", bufs=1) as wpool,
            tc.tile_pool(name="xpool", bufs=1) as xpool,
            tc.tile_pool(name="bpool", bufs=1) as bpool,
            tc.tile_pool(name="o0pool", bufs=1) as o0pool,
            tc.tile_pool(name="opool", bufs=2) as opool,
            tc.tile_pool(name="psum", bufs=8, space="PSUM") as psum_pool,
        ):
            # HAM warmup: dummy matmuls on memset tiles bridge the
            # DMA-starved gate window (~8-11 us) so real matmuls start at
            # the 2.4 GHz clock instead of paying the 3.4 us cold ramp
            dumx = xpool.tile([KT, MT], bf, name="dumx", tag="dumx")
            dumw = wpool.tile([KT, NT], bf, name="dumw", tag="dumw")
            nc.gpsimd.memset(dumx[:], 0)
            nc.gpsimd.memset(dumw[:], 0)
            dacc = psum_pool.tile([MT, NT], mybir.dt.float32, name="acc",
                                  tag="acc")
            # 8 dummies = 3.4 us of PE busy: exactly one HAM window, ending
            # right at first-data arrival (~11 us) with no idle gap
            for _ in range(8):
                nc.tensor.matmul(dacc[:], dumx[:], dumw[:],
                                 start=True, stop=True)

            # chunk-0 x k-slices as separate 128 KB tiles: the first matmul
            # gates on one slice, not the whole chunk (deps are per-tile)
            x0k = [xpool.tile([KT, CT], bf, name=f"x0k{k}", tag=f"x0k{k}")
                   for k in range(KTILES)]
            for k in range(KTILES):
                nc.scalar.dma_start(x0k[k][:], x0P[k])

            def x0s(k, m):  # stationary [128, MT] slice for chunk 0
                return x0k[k][:, m * MT:(m + 1) * MT]

            # n=0 weights k-sliced on the sync ring (gates chunk 0);
            # n=1..7 as whole 1 MB tiles behind them
            w0k = [wpool.tile([KT, NT], bf, name=f"w0k{k}", tag=f"w0k{k}")
                   for k in range(KTILES)]
            for k in range(KTILES):
                nc.sync.dma_start(w0k[k][:], w0P[k])
            wt = [wpool.tile([KT, KTILES * NT], bf, name=f"wt{n}",
                             tag=f"wt{n}") for n in range(1, NTILES)]
            for n in range(1, NTILES):
                nc.sync.dma_start(wt[n - 1][:], wP[n - 1])

            def ws(n, k):  # moving [128, NT] slice of expert weights
                if n == 0:
                    return w0k[k][:]
                return wt[n - 1][:, k * NT:(k + 1) * NT]

            # bias as per-n slices on the scalar ring BEHIND the x0k gate
            # slices: each drain gates on its own 256 KB slice, and the
            # early HBM window stays with the w stream
            bias_n = [bpool.tile([MT, NT], f32, name=f"bias{n}",
                                 tag=f"bias{n}") for n in range(NTILES)]
            for n in range(4):
                nc.scalar.dma_start(bias_n[n][:], biasP[n])

            # chunks 1-7 tiles; their DMAs are deferred (emitted after the
            # chunk-0 gate drain below) so the 7 MB x stream doesn't steal
            # HBM bandwidth from the latency-critical weight stream
            xt = [xpool.tile([KT, KTILES * CT], bf, name=f"xt{c}",
                             tag=f"xt{c}") for c in range(1, CHUNKS)]

            # ---- chunk 0: n-outer / k-inner, gated by wt[n] arrival ----
            o0 = [o0pool.tile([MT, DOUT], bf, name=f"o0_{m}", tag=f"o0_{m}")
                  for m in range(CMT)]
            xt_gate = None
            for n in range(NTILES):
                for m in range(CMT):
                    acc = psum_pool.tile([MT, NT], f32, name="acc", tag="acc")
                    for k in range(KTILES):
                        nc.tensor.matmul(
                            acc[:], x0s(k, m), ws(n, k),
                            start=(k == 0), stop=(k == KTILES - 1))
                    tt = nc.vector.tensor_add(
                        o0[m][:, n * NT:(n + 1) * NT], acc[:],
                        bias_n[n][:])
                    if n == 2 and m == CMT - 1:
                        xt_gate = tt
            for m in range(CMT):
                nc.sync.dma_start(y[m * MT:(m + 1) * MT, :], o0[m][:])

            # release the x chunk stream (and the late bias slices) once the
            # weight stream has had the HBM to itself (~30 us in, vs first
            # need at ~41 us for bias4 and ~67 us for xt[0])
            for n in range(4, NTILES):
                d = nc.scalar.dma_start(bias_n[n][:], biasP[n])
                add_dep_helper(d.ins, xt_gate.ins,
                               reason="defer late bias behind w stream")
            for i in range(CHUNKS - 1):
                d = nc.scalar.dma_start(xt[i][:], xP[i])
                add_dep_helper(d.ins, xt_gate.ins,
                               reason="defer x stream behind w stream")

            # ---- chunks 1-7: m-outer / k-mid / n-inner ----
            # stationary x[k,m] is shared by 8 matmuls into 8 PSUM banks
            for c in range(1, CHUNKS):
                xc = xt[c - 1]
                for m in range(CMT):
                    last_tile = c == CHUNKS - 1 and m == CMT - 1
                    row0 = c * CT + m * MT
                    if last_tile:
                        # n-outer so each bank drains right after its own
                        # k-group, with per-n 64 KB output DMAs: the kernel
                        # tail is one TT + one small DMA, not 8 TTs + 1 MB
                        ot = opool.tile([MT, DOUT], bf, name="ot", tag="ot")
                        for n in range(NTILES):
                            acc = psum_pool.tile([MT, NT], f32, name="acc",
                                                 tag="acc")
                            for k in range(KTILES):
                                xs = xc[:, k * CT + m * MT:
                                        k * CT + (m + 1) * MT]
                                nc.tensor.matmul(
                                    acc[:], xs, ws(n, k),
                                    start=(k == 0), stop=(k == KTILES - 1))
                            nc.vector.tensor_add(
                                ot[:, n * NT:(n + 1) * NT], acc[:],
                                bias_n[n][:])
                            nc.sync.dma_start(
                                y[row0:row0 + MT, n * NT:(n + 1) * NT],
                                ot[:, n * NT:(n + 1) * NT])
                        continue
                    accs = [psum_pool.tile([MT, NT], f32, name="acc",
                                           tag="acc") for _ in range(NTILES)]
                    for k in range(KTILES):
                        xs = xc[:, k * CT + m * MT:k * CT + (m + 1) * MT]
                        for n in range(NTILES):
                            nc.tensor.matmul(
                                accs[n][:], xs, ws(n, k),
                                start=(k == 0), stop=(k == KTILES - 1))
                    ot = opool.tile([MT, DOUT], bf, name="ot", tag="ot")
                    for n in range(NTILES):
                        nc.vector.tensor_add(
                            ot[:, n * NT:(n + 1) * NT], accs[n][:],
                            bias_n[n][:])
                    nc.sync.dma_start(y[row0:row0 + MT, :], ot[:])

    nc.compile()
    return nc


def _install_neff_cache():
    """Disk-cache walrus NEFF compiles keyed on the BIR bytes."""
    if _CACHE.get("neff_cache_installed"):
        return
    _CACHE["neff_cache_installed"] = True
    import hashlib
    import os
    import shutil

    import concourse.bass2jax as bass2jax

    cache_dir = "/root/.neff_bir_cache"
    os.makedirs(cache_dir, exist_ok=True)
    orig = bass2jax.compile_bir_kernel

    def cached_compile(ant_bir_str, tmpdir, neff_name="file.neff", **kw):
        key = hashlib.sha256(
            ant_bir_str if isinstance(ant_bir_str, bytes)
            else ant_bir_str.encode()).hexdigest()
        hit = os.path.join(cache_dir, key + ".neff")
        dst = os.path.join(tmpdir, neff_name)
        if os.path.exists(hit):
            shutil.copyfile(hit, dst)
            return dst
        out = orig(ant_bir_str, tmpdir, neff_name=neff_name, **kw)
        try:
            shutil.copyfile(out, hit)
        except OSError:
            pass
        return out

    bass2jax.compile_bir_kernel = cached_compile


def _get_nc():
    if "nc" not in _CACHE:
        _install_neff_cache()
        _CACHE["nc"] = _build_nc()
    return _CACHE["nc"]


def _pack_blocks(a2d, blocks, inner):
    """[blocks*inner, K*128] -> [blocks, 128, K*inner]:
    out[b, p, k*inner + j] = a2d[b*inner + j, k*128 + p]."""
    rows, cols = a2d.shape
    kb = cols // KT
    return np.ascontiguousarray(
        a2d.reshape(blocks, inner, kb, KT).transpose(0, 3, 2, 1)
        .reshape(blocks, KT, kb * inner))


def kernel(input, weight, bias, num_experts_per_token):
    from concourse.bass_utils import run_bass_kernel_spmd

    input = np.asarray(input, dtype=np.float32)
    weight = np.asarray(weight, dtype=np.float32)
    bias = np.ascontiguousarray(np.asarray(bias, dtype=np.float32))
    counts = np.asarray(num_experts_per_token).astype(np.int64)
    offsets = np.concatenate([[0], np.cumsum(counts)]).astype(np.int64)

    if counts.max() > TOKC:
        # capacity overflow (never hit with balanced routing): numpy fallback
        outs = []
        for i in range(E):
            xi = input[offsets[i]:offsets[i + 1]]
            outs.append(xi @ weight[i].T + bias[i])
        return np.concatenate(outs, axis=0)

    in_maps = []
    for i in range(E):
        xi = input[offsets[i]:offsets[i + 1]]  # [n_i, DIN]
        if xi.shape[0] < TOKC:
            xi = np.concatenate(
                [xi, np.zeros((TOKC - xi.shape[0], DIN), np.float32)], axis=0)
        xall = _pack_blocks(xi.astype(bfloat16), CHUNKS, CT)  # [8, 128, 4096]
        # chunk-0 cols are k*CT + j, so a k-slice is contiguous cols
        x0 = np.ascontiguousarray(
            xall[0].reshape(KT, KTILES, CT).transpose(1, 0, 2))
        wp = _pack_blocks(weight[i].astype(bfloat16), NTILES, NT)
        w0 = np.ascontiguousarray(
            wp[0].reshape(KT, KTILES, NT).transpose(1, 0, 2))
        bb = np.ascontiguousarray(
            np.broadcast_to(bias[i][None, :], (MT, DOUT))
            .reshape(MT, NTILES, NT).transpose(1, 0, 2))
        in_maps.append({"x0P": x0, "xP": np.ascontiguousarray(xall[1:]),
                        "w0P": w0, "wP": np.ascontiguousarray(wp[1:]),
                        "biasP": bb})

    nc = _get_nc()
    import os
    trace = bool(int(os.environ.get("KERNEL_TRACE", "0")))
    res = run_bass_kernel_spmd(nc, in_maps, core_ids=list(range(NCORES)),
                               trace=trace)
    _CACHE["last_result"] = res

    out = np.empty((T, DOUT), dtype=np.float32)
    pos = 0
    for i in range(E):
        n_i = int(counts[i])
        out[pos:pos + n_i] = res.results[i]["y"][:n_i].astype(np.float32)
        pos += n_i
    return out
